# revision 23
# baseline (speedup 1.0000x reference)
"""Multi-head self-attention block (B=2, S=2048, D=1024, H=16) on 8 TRN2 cores.

Sharding: 2-way data-parallel over batch x 4-way tensor-parallel over heads.
Core c handles batch b=c//4 with group rank g=c%4 (heads 4g..4g+4). After
attention, the per-head outputs x^T are exchanged with one small AllToAll
per q-quarter over the 4-core batch group (pure permutation, 4x fewer bytes
than reduce-scattering out-proj partials and no CC ALU work); each core then
runs the out-projection locally with the full W_out for the 128 output rows
it owns per quarter (rows 512q + 128g .. +128), adds the residual (with
b_out pre-folded on host) and LayerNorms. Collectives overlap the remaining
attention quarters; only quarter 3's exchange lands on the tail.

Score matmuls are packed block-diagonally (two 64-wide k-halves on the
128 partitions with Q duplicated) so the PE array runs fully active.

Self-contained: hardcodes all shapes; builds the Bass program once.
"""

import os
import sys

sys.path.insert(0, "/opt/trn_rl_repo")

import numpy as np
import ml_dtypes

import concourse.bass as bass
import concourse.tile as tile
from concourse import bacc, mybir
from concourse.bass_utils import run_bass_kernel_spmd

B, S, D, H = 2, 2048, 1024, 16
A = D // H  # 64
NCORES = 8
G = 4  # cores per batch group
HL = H // G  # local heads per core = 4
M_QK = 2 * HL * A  # 512 rows of Q_T+K_T per core
QB = S // G  # 512
EPS = 1e-3
GROUPS = [[0, 1, 2, 3], [4, 5, 6, 7]]

f32 = mybir.dt.float32
f32r = mybir.dt.float32r
bf16 = mybir.dt.bfloat16

AF = mybir.ActivationFunctionType
OP = mybir.AluOpType

_CACHE = {}


def _build():
    nc = bacc.Bacc("TRN2", target_bir_lowering=False, debug=False, num_devices=NCORES)

    # ---- I/O ----
    embT_d = nc.dram_tensor("embT", [D, S], bf16, kind="ExternalInput")
    embres_d = nc.dram_tensor("embres", [QB, D], f32, kind="ExternalInput")
    maskT_d = nc.dram_tensor("maskT", [S, S], bf16, kind="ExternalInput")
    wqk_d = nc.dram_tensor("wqk", [D, M_QK], bf16, kind="ExternalInput")
    wv_d = nc.dram_tensor("wv", [D, HL * A], bf16, kind="ExternalInput")
    bqk_d = nc.dram_tensor("bqk", [128, 4], f32, kind="ExternalInput")
    bv_d = nc.dram_tensor("bv", [1, HL * A], bf16, kind="ExternalInput")
    onesb_d = nc.dram_tensor("onesb", [1, 128], bf16, kind="ExternalInput")
    wout_d = nc.dram_tensor("wout", [128, 8, D], bf16, kind="ExternalInput")
    gamma_d = nc.dram_tensor("gamma", [1, D], bf16, kind="ExternalInput")
    beta_d = nc.dram_tensor("beta", [1, D], bf16, kind="ExternalInput")
    out_d = nc.dram_tensor("out", [QB, D], f32, kind="ExternalOutput")

    with tile.TileContext(nc) as tc:
        with (
            tc.tile_pool(name="big", bufs=1) as big,
            tc.tile_pool(name="persist", bufs=1) as persist,
            tc.tile_pool(name="probs", bufs=2) as probsp,
            tc.tile_pool(name="work", bufs=2) as work,
            tc.tile_pool(name="psA", bufs=2, space="PSUM") as psA,  # 1-bank f32 mm
            tc.tile_pool(name="psS", bufs=2, space="PSUM") as psS,  # scores (2 banks)
            tc.tile_pool(name="psB", bufs=2, space="PSUM") as psB,  # pv xT (1 bank)
            tc.tile_pool(name="dram", bufs=1, space="DRAM") as dram,
        ):
            # ---------- constants / small weights first (cheap, unblock evictions) ----------
            bqk_sb = persist.tile([128, 4], f32)
            nc.sync.dma_start(out=bqk_sb, in_=bqk_d[:, :])
            bv_sb = persist.tile([1, HL * A], bf16)
            nc.sync.dma_start(out=bv_sb, in_=bv_d[:, :])
            ones_b = persist.tile([1, 128], bf16)
            nc.sync.dma_start(out=ones_b, in_=onesb_d[:, :])

            # ---------- embT + wqk interleaved, s-quarter-major: the first
            # QKV matmul group (sc=0) only needs embT[:, :, 0:512] + wqk.
            embT_sb = big.tile([128, 8, S], bf16, tag="bigslot")
            wqk_sb = persist.tile([128, 8, M_QK], bf16)
            wv_sb = persist.tile([128, 8, HL * A], bf16)
            for kt in range(8):
                nc.sync.dma_start(
                    out=embT_sb[:, kt, 0:512], in_=embT_d[kt * 128 : (kt + 1) * 128, 0:512]
                )
                nc.sync.dma_start(out=wqk_sb[:, kt, :], in_=wqk_d[kt * 128 : (kt + 1) * 128, :])
            for kt in range(8):
                nc.sync.dma_start(
                    out=embT_sb[:, kt, 512:S], in_=embT_d[kt * 128 : (kt + 1) * 128, 512:S]
                )
            for kt in range(8):
                nc.sync.dma_start(out=wv_sb[:, kt, :], in_=wv_d[kt * 128 : (kt + 1) * 128, :])

            wout_sb = persist.tile([128, 8, D], bf16)
            eps_sb = persist.tile([128, 1], f32)
            nc.vector.memset(eps_sb, EPS)
            gammabc = persist.tile([128, D], bf16)
            betabc = persist.tile([128, D], bf16)

            # ---------- QKV projection ----------
            # Q duplicated on both partition halves: q2[p, h, s], p<64 and
            # p>=64 both hold Q_h[p % 64, s].
            q2_sb = persist.tile([128, HL, S], bf16)
            # K block-diag: k2[0:64, h, kb, 0:64] = K_h[a, 128 kb + m],
            # k2[64:128, h, kb, 64:128] = K_h[a, 128 kb + 64 + m], zeros off-diag.
            k2_sb = persist.tile([128, HL, 16, 128], bf16)
            nc.vector.memset(k2_sb, 0.0)

            for sc in range(4):  # sc-outer so the first group needs only embT sc=0
                for mc in range(4):  # 0: Q h0-1, 1: Q h2-3, 2: K h0-1, 3: K h2-3
                    ps = psA.tile([128, 512], f32, tag="aux")
                    for kt in range(8):
                        nc.tensor.matmul(
                            ps[:],
                            wqk_sb[:, kt, mc * 128 : (mc + 1) * 128],
                            embT_sb[:, kt, sc * 512 : (sc + 1) * 512],
                            start=(kt == 0),
                            stop=(kt == 7),
                        )
                    if mc < 2:  # Q: natural-half eviction per head
                        he, ho = 2 * mc, 2 * mc + 1
                        nc.scalar.activation(
                            out=q2_sb[0:64, he, sc * 512 : (sc + 1) * 512],
                            in_=ps[0:64, :],
                            func=AF.Identity,
                            bias=bqk_sb[0:64, mc : mc + 1],
                            scale=1.0,
                        )
                        nc.scalar.activation(
                            out=q2_sb[64:128, ho, sc * 512 : (sc + 1) * 512],
                            in_=ps[64:128, :],
                            func=AF.Identity,
                            bias=bqk_sb[64:128, mc : mc + 1],
                            scale=1.0,
                        )
                    else:  # K: stage, then scatter into the diag blocks via DMA
                        he, ho = 2 * (mc - 2), 2 * (mc - 2) + 1
                        kstage = work.tile([128, 512], bf16, tag="kstage", bufs=3)
                        nc.scalar.activation(
                            out=kstage[:],
                            in_=ps[:],
                            func=AF.Identity,
                            bias=bqk_sb[:, mc : mc + 1],
                            scale=1.0,
                        )
                        ks = kstage.rearrange("p (k t m) -> p k t m", k=4, m=64)
                        kslc = slice(sc * 4, (sc + 1) * 4)
                        nc.gpsimd.dma_start(
                            out=k2_sb[0:64, he, kslc, 0:64], in_=ks[0:64, :, 0, :]
                        )
                        nc.gpsimd.dma_start(
                            out=k2_sb[64:128, he, kslc, 64:128], in_=ks[0:64, :, 1, :]
                        )
                        nc.gpsimd.dma_start(
                            out=k2_sb[0:64, ho, kslc, 0:64], in_=ks[64:128, :, 0, :]
                        )
                        nc.gpsimd.dma_start(
                            out=k2_sb[64:128, ho, kslc, 64:128], in_=ks[64:128, :, 1, :]
                        )
            # duplicate Q across partition halves (SBUF->SBUF DMA moves partitions)
            for h in range(HL):
                if h % 2 == 0:
                    nc.gpsimd.dma_start(out=q2_sb[64:128, h, :], in_=q2_sb[0:64, h, :])
                else:
                    nc.gpsimd.dma_start(out=q2_sb[0:64, h, :], in_=q2_sb[64:128, h, :])

            # V: [s, (h, a+1)] bf16, ones column LAST per head (sumexp row trick)
            v_sb = persist.tile([128, 16, HL, 1 + A], bf16)
            nc.vector.memset(v_sb, 1.0)
            for st in range(16):
                ps = psA.tile([128, HL * A], f32, tag="aux")
                for kt in range(8):
                    nc.tensor.matmul(
                        ps[:],
                        embT_sb[:, kt, st * 128 : (st + 1) * 128],
                        wv_sb[:, kt, :],
                        start=(kt == 0),
                        stop=False,
                    )
                nc.tensor.matmul(ps[:], ones_b[:, :], bv_sb[:, :], start=False, stop=True)
                nc.vector.tensor_copy(
                    out=v_sb[:, st, :, 0:A],
                    in_=ps.rearrange("p (h a) -> p h a", h=HL),
                )

            # out-proj weights + LN constants issued late: needed only from the
            # first consume (~mid-kernel); keeps them out of the startup DMA rush
            for kc in range(8):
                nc.sync.dma_start(out=wout_sb[:, kc, :], in_=wout_d[:, kc, :])
            for t, dr in ((gammabc, gamma_d), (betabc, beta_d)):
                src = dr[:, :]
                bc = bass.AP(tensor=src.tensor, offset=src.offset, ap=[[0, 128], src.ap[1]])
                nc.sync.dma_start(out=t[:], in_=bc)

            # xT: [(a), head pair, s] — heads stacked two per 128 partitions
            xT_sb = persist.tile([128, 2, S], bf16)

            # ---------- attention, software-pipelined two units deep ----------
            def pv_mms(pu, kb0, kb1):
                if pu["ps_x"] is None:
                    pu["ps_x"] = psB.tile([65, 512], f32, name="ps_x", tag="pvx")
                for kb in range(kb0, kb1):
                    nc.tensor.matmul(
                        pu["ps_x"][:],
                        v_sb[:, kb, pu["h"], :],
                        pu["probs"][:, kb, :],
                        start=(kb == 0),
                        stop=(kb == 15),
                    )

            def normalize_evict(pu):
                pq, ph, pps_x = pu["q"], pu["h"], pu["ps_x"]
                qo = pq * 512
                # 1/sumexp: DVE reciprocal is 8 cyc/element/lane, so spread the
                # 512 sums over 64 partitions (8 per lane) before inverting,
                # then gather + broadcast back. DMA hops hide in the 2-unit lag.
                recip = work.tile([65, 512], f32r, tag="recip")
                with nc.allow_low_precision(reason="f32r is bitwise f32"):
                    nc.vector.tensor_copy(out=recip[64:65, :], in_=pps_x[64:65, :])
                se_d = dram.tile([1, 512], f32r, name="sed", tag="sed", bufs=2)
                nc.sync.dma_start(out=se_d[:, :], in_=recip[64:65, :])
                se_sp = work.tile([64, 2, 8], f32r, tag="sesp")
                nc.sync.dma_start(
                    out=se_sp[:, 0, :], in_=se_d.rearrange("o (p j) -> (o p) j", p=64)
                )
                with nc.allow_low_precision(reason="f32r is bitwise f32"):
                    nc.vector.reciprocal(se_sp[:, 1, :], se_sp[:, 0, :])
                recip_d = dram.tile([1, 512], f32r, name="recipd", tag="recipd", bufs=2)
                nc.sync.dma_start(
                    out=recip_d.rearrange("o (p j) -> (o p) j", p=64), in_=se_sp[:, 1, :]
                )
                src = recip_d[:, :]
                bc = bass.AP(tensor=src.tensor, offset=src.offset, ap=[[0, 64], src.ap[-1]])
                nc.sync.dma_start(out=recip[0:64, :], in_=bc)
                if ph % 2 == 0:
                    nc.vector.tensor_tensor(
                        xT_sb[0:64, ph // 2, qo : qo + 512],
                        pps_x[0:64, :],
                        recip[0:64, :],
                        OP.mult,
                    )
                else:
                    xodd = work.tile([64, 512], bf16, tag="xodd")
                    nc.vector.tensor_tensor(
                        xodd[:], pps_x[0:64, :], recip[0:64, :], OP.mult
                    )
                    nc.sync.dma_start(
                        out=xT_sb[64:128, ph // 2, qo : qo + 512], in_=xodd[:]
                    )

            a2a_out = {}
            er_tiles = {}
            # my group rank, runtime: selects my 128-row column block of the
            # gathered x^T when consuming
            g_dyn = nc.sync.partition_id() % 4

            def a2a_quarter(q, pairs=(0, 1)):
                # AllGather x^T within the 4-core group: each core contributes
                # its head-dims (1 or 2 pair-slabs) for all 512 rows of the
                # quarter; everyone receives the gathered dims and consumes
                # only its own 128-row column block. The collective-feeding
                # DMAs ride the idle GPSIMD SWDGE queue so they never sit
                # behind bulk loads in the sync HWDGE FIFO.
                qo = q * 512
                npair = len(pairs)
                key = f"ag{q}_{pairs[0]}"
                in_d = dram.tile([128 * npair, 512], bf16, name=f"i{key}", tag=f"i{key}")
                out_dd = dram.tile([512 * npair, 512], bf16, name=f"o{key}", tag=f"o{key}")
                for j, pr in enumerate(pairs):
                    nc.gpsimd.dma_start(
                        out=in_d[128 * j : 128 * (j + 1), :],
                        in_=xT_sb[:, pr, qo : qo + 512],
                    )
                nc.gpsimd.collective_compute(
                    "AllGather",
                    OP.bypass,
                    replica_groups=GROUPS,
                    ins=[in_d[:, :].opt()],
                    outs=[out_dd[:, :].opt()],
                )
                a2a_out.setdefault(q, []).append((pairs, out_dd))
                if q not in er_tiles:
                    er = work.tile([128, D], f32, tag="er")
                    nc.sync.dma_start(out=er[:], in_=embres_d[q * 128 : (q + 1) * 128, :])
                    er_tiles[q] = er

            def consume_quarter(q):
                # gathered x^T -> local out-proj (full W_out) -> residual+LN
                xg = work.tile([128, 8, 128], bf16, tag="xg", bufs=1)
                for pairs, out_dd in a2a_out.pop(q):
                    npair = len(pairs)
                    src = out_dd.rearrange("(r p) c -> p r c", p=128)
                    if npair == 2:
                        nc.sync.dma_start(
                            out=xg[:, :, :], in_=src[:, :, bass.ds(g_dyn * 128, 128)]
                        )
                    else:
                        xgv = xg.rearrange("p (k two) c -> p k two c", two=2)
                        nc.sync.dma_start(
                            out=xgv[:, :, pairs[0], :],
                            in_=src[:, :, bass.ds(g_dyn * 128, 128)],
                        )
                er = er_tiles.pop(q)
                y = work.tile([128, D], f32, tag="y")
                stats = work.tile([128, 2, nc.vector.BN_STATS_DIM], f32, tag="stats")
                for dh in range(2):
                    ps_o = psA.tile([128, 512], f32, tag="aux")
                    for kc in range(8):
                        nc.tensor.matmul(
                            ps_o[:],
                            xg[:, kc, :],
                            wout_sb[:, kc, dh * 512 : (dh + 1) * 512],
                            start=(kc == 0),
                            stop=(kc == 7),
                        )
                    hsl = slice(dh * 512, (dh + 1) * 512)
                    nc.vector.tensor_tensor(y[:, hsl], er[:, hsl], ps_o[:], OP.add)
                    nc.vector.bn_stats(out=stats[:, dh, :], in_=y[:, hsl])
                mv = work.tile([128, nc.vector.BN_AGGR_DIM], f32, tag="mv")
                nc.vector.bn_aggr(out=mv[:], in_=stats[:])
                rstd = work.tile([128, 1], f32, tag="rstd")
                nc.scalar.activation(
                    out=rstd[:], in_=mv[:, 1:2], func=AF.Sqrt, bias=eps_sb[:], scale=1.0
                )
                nc.vector.reciprocal(rstd[:], rstd[:])
                nc.vector.tensor_scalar(
                    y[:], y[:], mv[:, 0:1], rstd[:], OP.subtract, OP.mult
                )
                nc.vector.tensor_tensor(y[:], y[:], gammabc[:], OP.mult)
                nc.vector.tensor_tensor(y[:], y[:], betabc[:], OP.add)
                nc.sync.dma_start(out=out_d[q * 128 : (q + 1) * 128, :], in_=y[:])

            def finish(pu):
                normalize_evict(pu)
                q = pu["q"]
                if q in (0, 3):  # split first (skew absorber) + last (tail) gathers
                    if pu["h"] == 1:
                        a2a_quarter(q, pairs=(0,))
                    elif pu["h"] == 3:
                        a2a_quarter(q, pairs=(1,))
                elif pu["h"] == 3:
                    a2a_quarter(q)

            units = []
            mq = None
            for quarter in range(4):
                qoff = quarter * 512
                for h in range(4):
                    if h == 0:  # per-quarter mask slice, double-buffered
                        mq = work.tile([128, 16, 512], bf16, name="mq", tag="maskq")
                        for kb in range(16):
                            nc.sync.dma_start(
                                out=mq[:, kb, :],
                                in_=maskT_d[kb * 128 : (kb + 1) * 128, qoff : qoff + 512],
                            )
                    probs = probsp.tile([128, 16, 512], bf16, tag="probs")
                    unit = {"q": quarter, "h": h, "probs": probs, "ps_x": None, "mq": mq}
                    for j in range(8):  # kb pairs
                        ps_s = psS.tile([128, 2, 512], f32, tag="score")
                        for kk in range(2):
                            kb = 2 * j + kk
                            nc.tensor.matmul(
                                ps_s[:, kk, :],
                                k2_sb[:, h, kb, :],
                                q2_sb[:, h, qoff : qoff + 512],
                                start=True,
                                stop=True,
                            )
                        if units:
                            pv_mms(units[-1], 2 * j, 2 * j + 2)
                        nc.scalar.activation(
                            out=probs[:, 2 * j : 2 * j + 2, :],
                            in_=ps_s[:, :, :],
                            func=AF.Exp,
                            scale=0.125,
                        )
                        if j in (3, 7):  # mask applied in 8-kb batches
                            kb0 = 0 if j == 3 else 8
                            nc.vector.tensor_tensor(
                                probs[:, kb0 : kb0 + 8, :],
                                probs[:, kb0 : kb0 + 8, :],
                                mq[:, kb0 : kb0 + 8, :],
                                OP.mult,
                            )
                        if j == 1 and len(units) >= 2:
                            finish(units[-2])
                        if j == 5 and quarter >= 2 and h == 1:
                            consume_quarter(quarter - 2)
                    units.append(unit)
            finish(units[-2])
            pv_mms(units[-1], 0, 16)
            finish(units[-1])  # triggers AG(3, pair 1)
            consume_quarter(2)  # overlaps the final gather's latency
            consume_quarter(3)

    nc.compile()
    return nc


def _prep_inputs(embeddings, attention_mask, W_qkv, b_qkv, W_out, b_out, ln_gamma, ln_beta):
    emb = np.asarray(embeddings, dtype=np.float32)
    mask = np.asarray(attention_mask)
    W_qkv = np.asarray(W_qkv, dtype=np.float32)
    b_qkv = np.asarray(b_qkv, dtype=np.float32)
    W_out = np.asarray(W_out, dtype=np.float32)
    b_out = np.asarray(b_out, dtype=np.float32)
    gamma = np.asarray(ln_gamma, dtype=np.float32).reshape(1, D).astype(ml_dtypes.bfloat16)
    beta = np.asarray(ln_beta, dtype=np.float32).reshape(1, D).astype(ml_dtypes.bfloat16)

    woutF = np.ascontiguousarray(
        W_out.reshape(8, 128, D).transpose(1, 0, 2)
    ).astype(ml_dtypes.bfloat16)

    in_maps = []
    for c in range(NCORES):
        b = c // G
        g = c % G
        hs = g * HL * A  # 256g
        embT = np.ascontiguousarray(emb[b].T).astype(ml_dtypes.bfloat16)
        maskT = np.ascontiguousarray(mask[b].T).astype(ml_dtypes.bfloat16)
        wqk = np.ascontiguousarray(
            np.concatenate([W_qkv[:, hs : hs + 256], W_qkv[:, D + hs : D + hs + 256]], axis=1)
        ).astype(ml_dtypes.bfloat16)
        wv = np.ascontiguousarray(W_qkv[:, 2 * D + hs : 2 * D + hs + 256]).astype(
            ml_dtypes.bfloat16
        )
        bqk = np.concatenate([b_qkv[hs : hs + 256], b_qkv[D + hs : D + hs + 256]])
        bqk = np.ascontiguousarray(bqk.reshape(4, 128).T)
        bv = np.ascontiguousarray(
            b_qkv[2 * D + hs : 2 * D + hs + 256].reshape(1, 256)
        ).astype(ml_dtypes.bfloat16)
        embres = np.concatenate(
            [emb[b, 512 * q + 128 * g : 512 * q + 128 * g + 128, :] for q in range(4)],
            axis=0,
        ) + b_out.reshape(1, D)
        in_maps.append(
            {
                "embT": embT,
                "embres": np.ascontiguousarray(embres),
                "maskT": maskT,
                "wqk": wqk,
                "wv": wv,
                "bqk": bqk,
                "bv": bv,
                "onesb": np.ones((1, 128), dtype=ml_dtypes.bfloat16),
                "wout": woutF,
                "gamma": gamma,
                "beta": beta,
            }
        )
    return in_maps


def _run(inputs, trace=False, **kw):
    if "nc" not in _CACHE:
        _CACHE["nc"] = _build()
    nc = _CACHE["nc"]
    in_maps = _prep_inputs(**inputs)
    res = run_bass_kernel_spmd(nc, in_maps, list(range(NCORES)), trace=trace, **kw)
    out = np.empty((B, S, D), dtype=np.float32)
    for c in range(NCORES):
        b, g = c // G, c % G
        for q in range(4):
            out[b, 512 * q + 128 * g : 512 * q + 128 * g + 128, :] = res.results[c][
                "out"
            ][128 * q : 128 * (q + 1), :]
    return out, res


def kernel(**inputs):
    out, _ = _run(inputs, trace=False)
    return out


# revision 27
# speedup vs baseline: 1.1526x; 1.1526x over previous
"""Multi-head self-attention block (B=2, S=2048, D=1024, H=16) on 8 TRN2 cores.

Sharding: 2-way data-parallel over batch x 4-way tensor-parallel over heads.
Core c handles batch b=c//4 with group rank g=c%4 (heads 4g..4g+4). After
attention, the per-head outputs x^T are exchanged with one small AllToAll
per q-quarter over the 4-core batch group (pure permutation, 4x fewer bytes
than reduce-scattering out-proj partials and no CC ALU work); each core then
runs the out-projection locally with the full W_out for the 128 output rows
it owns per quarter (rows 512q + 128g .. +128), adds the residual (with
b_out pre-folded on host) and LayerNorms. Collectives overlap the remaining
attention quarters; only quarter 3's exchange lands on the tail.

Score matmuls are packed block-diagonally (two 64-wide k-halves on the
128 partitions with Q duplicated) so the PE array runs fully active.

Self-contained: hardcodes all shapes; builds the Bass program once.
"""

import os
import sys

sys.path.insert(0, "/opt/trn_rl_repo")

import numpy as np
import ml_dtypes

import concourse.bass as bass
import concourse.tile as tile
from concourse import bacc, mybir
from concourse.bass_utils import run_bass_kernel_spmd

B, S, D, H = 2, 2048, 1024, 16
A = D // H  # 64
NCORES = 8
G = 4  # cores per batch group
HL = H // G  # local heads per core = 4
M_QK = 2 * HL * A  # 512 rows of Q_T+K_T per core
QB = S // G  # 512
EPS = 1e-3
GROUPS = [[0, 1, 2, 3], [4, 5, 6, 7]]

f32 = mybir.dt.float32
f32r = mybir.dt.float32r
bf16 = mybir.dt.bfloat16

AF = mybir.ActivationFunctionType
OP = mybir.AluOpType

_CACHE = {}


def _build():
    nc = bacc.Bacc("TRN2", target_bir_lowering=False, debug=False, num_devices=NCORES)

    # ---- I/O ----
    embT_d = nc.dram_tensor("embT", [D, S], bf16, kind="ExternalInput")
    embres_d = nc.dram_tensor("embres", [QB, D], f32, kind="ExternalInput")
    maskT_d = nc.dram_tensor("maskT", [S, S], bf16, kind="ExternalInput")
    wqk_d = nc.dram_tensor("wqk", [D, M_QK], bf16, kind="ExternalInput")
    wv_d = nc.dram_tensor("wv", [D, HL * A], bf16, kind="ExternalInput")
    bqk_d = nc.dram_tensor("bqk", [128, 4], f32, kind="ExternalInput")
    bv_d = nc.dram_tensor("bv", [1, HL * A], bf16, kind="ExternalInput")
    onesb_d = nc.dram_tensor("onesb", [1, 128], bf16, kind="ExternalInput")
    wout_d = nc.dram_tensor("wout", [128, 8, D], bf16, kind="ExternalInput")
    gamma_d = nc.dram_tensor("gamma", [1, D], bf16, kind="ExternalInput")
    beta_d = nc.dram_tensor("beta", [1, D], bf16, kind="ExternalInput")
    out_d = nc.dram_tensor("out", [QB, D], f32, kind="ExternalOutput")

    with tile.TileContext(nc) as tc:
        with (
            tc.tile_pool(name="big", bufs=1) as big,
            tc.tile_pool(name="persist", bufs=1) as persist,
            tc.tile_pool(name="probs", bufs=2) as probsp,
            tc.tile_pool(name="work", bufs=2) as work,
            tc.tile_pool(name="psA", bufs=2, space="PSUM") as psA,  # 1-bank f32 mm
            tc.tile_pool(name="psS", bufs=2, space="PSUM") as psS,  # scores (2 banks)
            tc.tile_pool(name="psB", bufs=2, space="PSUM") as psB,  # pv xT (1 bank)
            tc.tile_pool(name="dram", bufs=1, space="DRAM") as dram,
        ):
            # ---------- constants / small weights first (cheap, unblock evictions) ----------
            bqk_sb = persist.tile([128, 4], f32)
            nc.sync.dma_start(out=bqk_sb, in_=bqk_d[:, :])
            bv_sb = persist.tile([1, HL * A], bf16)
            nc.sync.dma_start(out=bv_sb, in_=bv_d[:, :])
            ones_b = persist.tile([1, 128], bf16)
            nc.sync.dma_start(out=ones_b, in_=onesb_d[:, :])

            # ---------- embT + wqk interleaved, s-quarter-major: the first
            # QKV matmul group (sc=0) only needs embT[:, :, 0:512] + wqk.
            embT_sb = big.tile([128, 8, S], bf16, tag="bigslot")
            wqk_sb = persist.tile([128, 8, M_QK], bf16)
            wv_sb = persist.tile([128, 8, HL * A], bf16)
            for kt in range(8):
                nc.sync.dma_start(
                    out=embT_sb[:, kt, 0:512], in_=embT_d[kt * 128 : (kt + 1) * 128, 0:512]
                )
                nc.sync.dma_start(out=wqk_sb[:, kt, :], in_=wqk_d[kt * 128 : (kt + 1) * 128, :])
            for kt in range(8):
                nc.sync.dma_start(
                    out=embT_sb[:, kt, 512:S], in_=embT_d[kt * 128 : (kt + 1) * 128, 512:S]
                )
            for kt in range(8):
                nc.sync.dma_start(out=wv_sb[:, kt, :], in_=wv_d[kt * 128 : (kt + 1) * 128, :])

            wout_sb = persist.tile([128, 8, D], bf16)
            eps_sb = persist.tile([128, 1], f32)
            nc.vector.memset(eps_sb, EPS)
            gammabc = persist.tile([128, D], bf16)
            betabc = persist.tile([128, D], bf16)

            # ---------- QKV projection ----------
            # Q duplicated on both partition halves: q2[p, h, s], p<64 and
            # p>=64 both hold Q_h[p % 64, s].
            q2_sb = persist.tile([128, HL, S], bf16)
            # K block-diag: k2[0:64, h, kb, 0:64] = K_h[a, 128 kb + m],
            # k2[64:128, h, kb, 64:128] = K_h[a, 128 kb + 64 + m], zeros off-diag.
            k2_sb = persist.tile([128, HL, 16, 128], bf16)
            nc.vector.memset(k2_sb, 0.0)

            for sc in range(4):  # sc-outer so the first group needs only embT sc=0
                for mc in range(4):  # 0: Q h0-1, 1: Q h2-3, 2: K h0-1, 3: K h2-3
                    ps = psA.tile([128, 512], f32, tag="aux")
                    for kt in range(8):
                        nc.tensor.matmul(
                            ps[:],
                            wqk_sb[:, kt, mc * 128 : (mc + 1) * 128],
                            embT_sb[:, kt, sc * 512 : (sc + 1) * 512],
                            start=(kt == 0),
                            stop=(kt == 7),
                        )
                    if mc < 2:  # Q: natural-half eviction per head
                        he, ho = 2 * mc, 2 * mc + 1
                        nc.scalar.activation(
                            out=q2_sb[0:64, he, sc * 512 : (sc + 1) * 512],
                            in_=ps[0:64, :],
                            func=AF.Identity,
                            bias=bqk_sb[0:64, mc : mc + 1],
                            scale=1.0,
                        )
                        nc.scalar.activation(
                            out=q2_sb[64:128, ho, sc * 512 : (sc + 1) * 512],
                            in_=ps[64:128, :],
                            func=AF.Identity,
                            bias=bqk_sb[64:128, mc : mc + 1],
                            scale=1.0,
                        )
                    else:  # K: stage, then scatter into the diag blocks via DMA
                        he, ho = 2 * (mc - 2), 2 * (mc - 2) + 1
                        kstage = work.tile([128, 512], bf16, tag="kstage", bufs=3)
                        nc.scalar.activation(
                            out=kstage[:],
                            in_=ps[:],
                            func=AF.Identity,
                            bias=bqk_sb[:, mc : mc + 1],
                            scale=1.0,
                        )
                        ks = kstage.rearrange("p (k t m) -> p k t m", k=4, m=64)
                        kslc = slice(sc * 4, (sc + 1) * 4)
                        nc.gpsimd.dma_start(
                            out=k2_sb[0:64, he, kslc, 0:64], in_=ks[0:64, :, 0, :]
                        )
                        nc.gpsimd.dma_start(
                            out=k2_sb[64:128, he, kslc, 64:128], in_=ks[0:64, :, 1, :]
                        )
                        nc.gpsimd.dma_start(
                            out=k2_sb[0:64, ho, kslc, 0:64], in_=ks[64:128, :, 0, :]
                        )
                        nc.gpsimd.dma_start(
                            out=k2_sb[64:128, ho, kslc, 64:128], in_=ks[64:128, :, 1, :]
                        )
            # duplicate Q across partition halves (SBUF->SBUF DMA moves partitions)
            for h in range(HL):
                if h % 2 == 0:
                    nc.gpsimd.dma_start(out=q2_sb[64:128, h, :], in_=q2_sb[0:64, h, :])
                else:
                    nc.gpsimd.dma_start(out=q2_sb[0:64, h, :], in_=q2_sb[64:128, h, :])

            # V: [s, (h, a+1)] bf16, ones column LAST per head (sumexp row trick)
            v_sb = persist.tile([128, 16, HL, 1 + A], bf16)
            nc.vector.memset(v_sb, 1.0)
            for st in range(16):
                ps = psA.tile([128, HL * A], f32, tag="aux")
                for kt in range(8):
                    nc.tensor.matmul(
                        ps[:],
                        embT_sb[:, kt, st * 128 : (st + 1) * 128],
                        wv_sb[:, kt, :],
                        start=(kt == 0),
                        stop=False,
                    )
                nc.tensor.matmul(ps[:], ones_b[:, :], bv_sb[:, :], start=False, stop=True)
                nc.vector.tensor_copy(
                    out=v_sb[:, st, :, 0:A],
                    in_=ps.rearrange("p (h a) -> p h a", h=HL),
                )

            # out-proj weights + LN constants issued late: needed only from the
            # first consume (~mid-kernel); keeps them out of the startup DMA rush
            for kc in range(8):
                nc.scalar.dma_start(out=wout_sb[:, kc, :], in_=wout_d[:, kc, :])
            for t, dr in ((gammabc, gamma_d), (betabc, beta_d)):
                src = dr[:, :]
                bc = bass.AP(tensor=src.tensor, offset=src.offset, ap=[[0, 128], src.ap[1]])
                nc.scalar.dma_start(out=t[:], in_=bc)

            # xT: [(a), head pair, s] — heads stacked two per 128 partitions
            xT_sb = persist.tile([128, 2, S], bf16)

            # ---------- attention, software-pipelined two units deep ----------
            def pv_mms(pu, kb0, kb1):
                if pu["ps_x"] is None:
                    pu["ps_x"] = psB.tile([65, 512], f32, name="ps_x", tag="pvx")
                for kb in range(kb0, kb1):
                    nc.tensor.matmul(
                        pu["ps_x"][:],
                        v_sb[:, kb, pu["h"], :],
                        pu["probs"][:, kb, :],
                        start=(kb == 0),
                        stop=(kb == 15),
                    )

            def normalize_evict(pu):
                pq, ph, pps_x = pu["q"], pu["h"], pu["ps_x"]
                qo = pq * 512
                # 1/sumexp: DVE reciprocal is 8 cyc/element/lane, so spread the
                # 512 sums over 64 partitions (8 per lane) before inverting,
                # then gather + broadcast back. DMA hops hide in the 2-unit lag.
                recip = work.tile([65, 512], f32r, tag="recip")
                with nc.allow_low_precision(reason="f32r is bitwise f32"):
                    nc.vector.tensor_copy(out=recip[64:65, :], in_=pps_x[64:65, :])
                se_d = dram.tile([1, 512], f32r, name="sed", tag="sed", bufs=2)
                nc.sync.dma_start(out=se_d[:, :], in_=recip[64:65, :])
                se_sp = work.tile([64, 2, 8], f32r, tag="sesp")
                nc.sync.dma_start(
                    out=se_sp[:, 0, :], in_=se_d.rearrange("o (p j) -> (o p) j", p=64)
                )
                with nc.allow_low_precision(reason="f32r is bitwise f32"):
                    nc.vector.reciprocal(se_sp[:, 1, :], se_sp[:, 0, :])
                recip_d = dram.tile([1, 512], f32r, name="recipd", tag="recipd", bufs=2)
                nc.sync.dma_start(
                    out=recip_d.rearrange("o (p j) -> (o p) j", p=64), in_=se_sp[:, 1, :]
                )
                src = recip_d[:, :]
                bc = bass.AP(tensor=src.tensor, offset=src.offset, ap=[[0, 64], src.ap[-1]])
                nc.sync.dma_start(out=recip[0:64, :], in_=bc)
                if ph % 2 == 0:
                    nc.vector.tensor_tensor(
                        xT_sb[0:64, ph // 2, qo : qo + 512],
                        pps_x[0:64, :],
                        recip[0:64, :],
                        OP.mult,
                    )
                else:
                    xodd = work.tile([64, 512], bf16, tag="xodd")
                    nc.vector.tensor_tensor(
                        xodd[:], pps_x[0:64, :], recip[0:64, :], OP.mult
                    )
                    nc.sync.dma_start(
                        out=xT_sb[64:128, ph // 2, qo : qo + 512], in_=xodd[:]
                    )

            a2a_out = {}
            er_tiles = {}
            # my group rank, runtime: selects my 128-row column block of the
            # gathered x^T when consuming
            g_dyn = nc.sync.partition_id() % 4

            def a2a_quarter(q, pairs=(0, 1)):
                # AllGather x^T within the 4-core group: each core contributes
                # its head-dims (1 or 2 pair-slabs) for all 512 rows of the
                # quarter; everyone receives the gathered dims and consumes
                # only its own 128-row column block. The collective-feeding
                # DMAs ride the idle GPSIMD SWDGE queue so they never sit
                # behind bulk loads in the sync HWDGE FIFO.
                qo = q * 512
                npair = len(pairs)
                key = f"ag{q}_{pairs[0]}"
                in_d = dram.tile([128 * npair, 512], bf16, name=f"i{key}", tag=f"i{key}")
                out_dd = dram.tile([512 * npair, 512], bf16, name=f"o{key}", tag=f"o{key}")
                for j, pr in enumerate(pairs):
                    nc.gpsimd.dma_start(
                        out=in_d[128 * j : 128 * (j + 1), :],
                        in_=xT_sb[:, pr, qo : qo + 512],
                    )
                nc.gpsimd.collective_compute(
                    "AllGather",
                    OP.bypass,
                    replica_groups=GROUPS,
                    ins=[in_d[:, :].opt()],
                    outs=[out_dd[:, :].opt()],
                )
                a2a_out.setdefault(q, []).append((pairs, out_dd))
                if q not in er_tiles:
                    er = work.tile([128, D], f32, tag="er")
                    nc.scalar.dma_start(out=er[:], in_=embres_d[q * 128 : (q + 1) * 128, :])
                    er_tiles[q] = er

            def consume_quarter(q):
                # gathered x^T -> local out-proj (full W_out) -> residual+LN
                xg = work.tile([128, 8, 128], bf16, tag="xg", bufs=1)
                for pairs, out_dd in a2a_out.pop(q):
                    npair = len(pairs)
                    src = out_dd.rearrange("(r p) c -> p r c", p=128)
                    if npair == 2:
                        nc.sync.dma_start(
                            out=xg[:, :, :], in_=src[:, :, bass.ds(g_dyn * 128, 128)]
                        )
                    else:
                        xgv = xg.rearrange("p (k two) c -> p k two c", two=2)
                        nc.sync.dma_start(
                            out=xgv[:, :, pairs[0], :],
                            in_=src[:, :, bass.ds(g_dyn * 128, 128)],
                        )
                er = er_tiles.pop(q)
                y = work.tile([128, D], f32, tag="y")
                stats = work.tile([128, 2, nc.vector.BN_STATS_DIM], f32, tag="stats")
                for dh in range(2):
                    ps_o = psA.tile([128, 512], f32, tag="aux")
                    for kc in range(8):
                        nc.tensor.matmul(
                            ps_o[:],
                            xg[:, kc, :],
                            wout_sb[:, kc, dh * 512 : (dh + 1) * 512],
                            start=(kc == 0),
                            stop=(kc == 7),
                        )
                    hsl = slice(dh * 512, (dh + 1) * 512)
                    nc.vector.tensor_tensor(y[:, hsl], er[:, hsl], ps_o[:], OP.add)
                    nc.vector.bn_stats(out=stats[:, dh, :], in_=y[:, hsl])
                mv = work.tile([128, nc.vector.BN_AGGR_DIM], f32, tag="mv")
                nc.vector.bn_aggr(out=mv[:], in_=stats[:])
                rstd = work.tile([128, 1], f32, tag="rstd")
                nc.scalar.activation(
                    out=rstd[:], in_=mv[:, 1:2], func=AF.Sqrt, bias=eps_sb[:], scale=1.0
                )
                nc.vector.reciprocal(rstd[:], rstd[:])
                nc.vector.tensor_scalar(
                    y[:], y[:], mv[:, 0:1], rstd[:], OP.subtract, OP.mult
                )
                nc.vector.tensor_tensor(y[:], y[:], gammabc[:], OP.mult)
                nc.vector.tensor_tensor(y[:], y[:], betabc[:], OP.add)
                nc.sync.dma_start(out=out_d[q * 128 : (q + 1) * 128, :], in_=y[:])

            def finish(pu):
                normalize_evict(pu)
                q = pu["q"]
                if q == 3:  # split the last gather by head pair: shorter tail
                    if pu["h"] == 1:
                        a2a_quarter(3, pairs=(0,))
                    elif pu["h"] == 3:
                        a2a_quarter(3, pairs=(1,))
                elif pu["h"] == 3:
                    a2a_quarter(q)

            units = []
            mq = None
            for quarter in range(4):
                qoff = quarter * 512
                for h in range(4):
                    if h == 0:  # per-quarter mask slice, double-buffered.
                        # Loaded via the ACT HWDGE ring: bulk, dependency-free
                        # DMAs stay out of the sync ring so they never
                        # head-block the small dependency-paced transfers.
                        mq = work.tile([128, 16, 512], bf16, name="mq", tag="maskq")
                        for kb in range(16):
                            nc.scalar.dma_start(
                                out=mq[:, kb, :],
                                in_=maskT_d[kb * 128 : (kb + 1) * 128, qoff : qoff + 512],
                            )
                    probs = probsp.tile([128, 16, 512], bf16, tag="probs")
                    unit = {"q": quarter, "h": h, "probs": probs, "ps_x": None, "mq": mq}
                    for j in range(8):  # kb pairs
                        ps_s = psS.tile([128, 2, 512], f32, tag="score")
                        for kk in range(2):
                            kb = 2 * j + kk
                            nc.tensor.matmul(
                                ps_s[:, kk, :],
                                k2_sb[:, h, kb, :],
                                q2_sb[:, h, qoff : qoff + 512],
                                start=True,
                                stop=True,
                            )
                        if units:
                            pv_mms(units[-1], 2 * j, 2 * j + 2)
                        nc.scalar.activation(
                            out=probs[:, 2 * j : 2 * j + 2, :],
                            in_=ps_s[:, :, :],
                            func=AF.Exp,
                            scale=0.125,
                        )
                        if j in (3, 7):  # mask applied in 8-kb batches
                            kb0 = 0 if j == 3 else 8
                            nc.vector.tensor_tensor(
                                probs[:, kb0 : kb0 + 8, :],
                                probs[:, kb0 : kb0 + 8, :],
                                mq[:, kb0 : kb0 + 8, :],
                                OP.mult,
                            )
                        if j == 1 and len(units) >= 2:
                            finish(units[-2])
                        if j == 5 and quarter >= 2 and h == 1:
                            consume_quarter(quarter - 2)
                    units.append(unit)
            finish(units[-2])
            pv_mms(units[-1], 0, 16)
            finish(units[-1])  # triggers AG(3, pair 1)
            consume_quarter(2)  # overlaps the final gather's latency
            consume_quarter(3)

    nc.compile()
    return nc


def _prep_inputs(embeddings, attention_mask, W_qkv, b_qkv, W_out, b_out, ln_gamma, ln_beta):
    emb = np.asarray(embeddings, dtype=np.float32)
    mask = np.asarray(attention_mask)
    W_qkv = np.asarray(W_qkv, dtype=np.float32)
    b_qkv = np.asarray(b_qkv, dtype=np.float32)
    W_out = np.asarray(W_out, dtype=np.float32)
    b_out = np.asarray(b_out, dtype=np.float32)
    gamma = np.asarray(ln_gamma, dtype=np.float32).reshape(1, D).astype(ml_dtypes.bfloat16)
    beta = np.asarray(ln_beta, dtype=np.float32).reshape(1, D).astype(ml_dtypes.bfloat16)

    woutF = np.ascontiguousarray(
        W_out.reshape(8, 128, D).transpose(1, 0, 2)
    ).astype(ml_dtypes.bfloat16)

    in_maps = []
    for c in range(NCORES):
        b = c // G
        g = c % G
        hs = g * HL * A  # 256g
        embT = np.ascontiguousarray(emb[b].T).astype(ml_dtypes.bfloat16)
        maskT = np.ascontiguousarray(mask[b].T).astype(ml_dtypes.bfloat16)
        wqk = np.ascontiguousarray(
            np.concatenate([W_qkv[:, hs : hs + 256], W_qkv[:, D + hs : D + hs + 256]], axis=1)
        ).astype(ml_dtypes.bfloat16)
        wv = np.ascontiguousarray(W_qkv[:, 2 * D + hs : 2 * D + hs + 256]).astype(
            ml_dtypes.bfloat16
        )
        bqk = np.concatenate([b_qkv[hs : hs + 256], b_qkv[D + hs : D + hs + 256]])
        bqk = np.ascontiguousarray(bqk.reshape(4, 128).T)
        bv = np.ascontiguousarray(
            b_qkv[2 * D + hs : 2 * D + hs + 256].reshape(1, 256)
        ).astype(ml_dtypes.bfloat16)
        embres = np.concatenate(
            [emb[b, 512 * q + 128 * g : 512 * q + 128 * g + 128, :] for q in range(4)],
            axis=0,
        ) + b_out.reshape(1, D)
        in_maps.append(
            {
                "embT": embT,
                "embres": np.ascontiguousarray(embres),
                "maskT": maskT,
                "wqk": wqk,
                "wv": wv,
                "bqk": bqk,
                "bv": bv,
                "onesb": np.ones((1, 128), dtype=ml_dtypes.bfloat16),
                "wout": woutF,
                "gamma": gamma,
                "beta": beta,
            }
        )
    return in_maps


def _run(inputs, trace=False, **kw):
    if "nc" not in _CACHE:
        _CACHE["nc"] = _build()
    nc = _CACHE["nc"]
    in_maps = _prep_inputs(**inputs)
    res = run_bass_kernel_spmd(nc, in_maps, list(range(NCORES)), trace=trace, **kw)
    out = np.empty((B, S, D), dtype=np.float32)
    for c in range(NCORES):
        b, g = c // G, c % G
        for q in range(4):
            out[b, 512 * q + 128 * g : 512 * q + 128 * g + 128, :] = res.results[c][
                "out"
            ][128 * q : 128 * (q + 1), :]
    return out, res


def kernel(**inputs):
    out, _ = _run(inputs, trace=False)
    return out


# revision 32
# speedup vs baseline: 1.2297x; 1.0669x over previous
"""Multi-head self-attention block (B=2, S=2048, D=1024, H=16) on 8 TRN2 cores.

Sharding: 2-way data-parallel over batch x 4-way tensor-parallel over heads.
Core c handles batch b=c//4 with group rank g=c%4 (heads 4g..4g+4). After
attention, the per-head outputs x^T are exchanged with one small AllToAll
per q-quarter over the 4-core batch group (pure permutation, 4x fewer bytes
than reduce-scattering out-proj partials and no CC ALU work); each core then
runs the out-projection locally with the full W_out for the 128 output rows
it owns per quarter (rows 512q + 128g .. +128), adds the residual (with
b_out pre-folded on host) and LayerNorms. Collectives overlap the remaining
attention quarters; only quarter 3's exchange lands on the tail.

Score matmuls are packed block-diagonally (two 64-wide k-halves on the
128 partitions with Q duplicated) so the PE array runs fully active.

Self-contained: hardcodes all shapes; builds the Bass program once.
"""

import os
import sys

sys.path.insert(0, "/opt/trn_rl_repo")

import numpy as np
import ml_dtypes

import concourse.bass as bass
import concourse.tile as tile
from concourse import bacc, mybir
from concourse.bass_utils import run_bass_kernel_spmd

B, S, D, H = 2, 2048, 1024, 16
A = D // H  # 64
NCORES = 8
G = 4  # cores per batch group
HL = H // G  # local heads per core = 4
M_QK = 2 * HL * A  # 512 rows of Q_T+K_T per core
QB = S // G  # 512
EPS = 1e-3
GROUPS = [[0, 1, 2, 3], [4, 5, 6, 7]]

f32 = mybir.dt.float32
f32r = mybir.dt.float32r
bf16 = mybir.dt.bfloat16

AF = mybir.ActivationFunctionType
OP = mybir.AluOpType

_CACHE = {}


def _build():
    nc = bacc.Bacc("TRN2", target_bir_lowering=False, debug=False, num_devices=NCORES)

    # ---- I/O ----
    embT_d = nc.dram_tensor("embT", [D, S], bf16, kind="ExternalInput")
    embres_d = nc.dram_tensor("embres", [QB, D], f32, kind="ExternalInput")
    maskT_d = nc.dram_tensor("maskT", [S, S], bf16, kind="ExternalInput")
    wqk_d = nc.dram_tensor("wqk", [D, M_QK], bf16, kind="ExternalInput")
    wv_d = nc.dram_tensor("wv", [D, HL * A], bf16, kind="ExternalInput")
    bqk_d = nc.dram_tensor("bqk", [128, 4], f32, kind="ExternalInput")
    bv_d = nc.dram_tensor("bv", [1, HL * A], bf16, kind="ExternalInput")
    onesb_d = nc.dram_tensor("onesb", [1, 128], bf16, kind="ExternalInput")
    wout_d = nc.dram_tensor("wout", [128, 8, D], bf16, kind="ExternalInput")
    gamma_d = nc.dram_tensor("gamma", [1, D], bf16, kind="ExternalInput")
    beta_d = nc.dram_tensor("beta", [1, D], bf16, kind="ExternalInput")
    out_d = nc.dram_tensor("out", [QB, D], f32, kind="ExternalOutput")

    with tile.TileContext(nc) as tc:
        with (
            tc.tile_pool(name="big", bufs=1) as big,
            tc.tile_pool(name="persist", bufs=1) as persist,
            tc.tile_pool(name="probs", bufs=2) as probsp,
            tc.tile_pool(name="work", bufs=2) as work,
            tc.tile_pool(name="psA", bufs=2, space="PSUM") as psA,  # 1-bank f32 mm
            tc.tile_pool(name="psS", bufs=2, space="PSUM") as psS,  # scores (2 banks)
            tc.tile_pool(name="psB", bufs=2, space="PSUM") as psB,  # pv xT (1 bank)
            tc.tile_pool(name="dram", bufs=1, space="DRAM") as dram,
        ):
            # ---------- constants / small weights first (cheap, unblock evictions) ----------
            bqk_sb = persist.tile([128, 4], f32)
            nc.sync.dma_start(out=bqk_sb, in_=bqk_d[:, :])
            bv_sb = persist.tile([1, HL * A], bf16)
            nc.sync.dma_start(out=bv_sb, in_=bv_d[:, :])
            ones_b = persist.tile([1, 128], bf16)
            nc.sync.dma_start(out=ones_b, in_=onesb_d[:, :])

            # ---------- embT + wqk interleaved, s-quarter-major: the first
            # QKV matmul group (sc=0) only needs embT[:, :, 0:512] + wqk.
            embT_sb = big.tile([128, 8, S], bf16, tag="bigslot")
            wqk_sb = persist.tile([128, 8, M_QK], bf16)
            wv_sb = persist.tile([128, 8, HL * A], bf16)
            for kt in range(8):
                nc.sync.dma_start(
                    out=embT_sb[:, kt, 0:512], in_=embT_d[kt * 128 : (kt + 1) * 128, 0:512]
                )
                nc.sync.dma_start(out=wqk_sb[:, kt, :], in_=wqk_d[kt * 128 : (kt + 1) * 128, :])
            for kt in range(8):
                nc.sync.dma_start(
                    out=embT_sb[:, kt, 512:S], in_=embT_d[kt * 128 : (kt + 1) * 128, 512:S]
                )
            for kt in range(8):
                nc.sync.dma_start(out=wv_sb[:, kt, :], in_=wv_d[kt * 128 : (kt + 1) * 128, :])

            wout_sb = persist.tile([128, 8, D], bf16)
            eps_sb = persist.tile([128, 1], f32)
            nc.vector.memset(eps_sb, EPS)
            gammabc = persist.tile([128, D], bf16)
            betabc = persist.tile([128, D], bf16)

            # ---------- QKV projection ----------
            # Q duplicated on both partition halves: q2[p, h, s], p<64 and
            # p>=64 both hold Q_h[p % 64, s].
            q2_sb = persist.tile([128, HL, S], bf16)
            # K block-diag: k2[0:64, h, kb, 0:64] = K_h[a, 128 kb + m],
            # k2[64:128, h, kb, 64:128] = K_h[a, 128 kb + 64 + m], zeros off-diag.
            k2_sb = persist.tile([128, HL, 16, 128], bf16)
            nc.vector.memset(k2_sb, 0.0)

            for sc in range(4):  # sc-outer so the first group needs only embT sc=0
                for mc in range(4):  # 0: Q h0-1, 1: Q h2-3, 2: K h0-1, 3: K h2-3
                    ps = psA.tile([128, 512], f32, tag="aux")
                    for kt in range(8):
                        nc.tensor.matmul(
                            ps[:],
                            wqk_sb[:, kt, mc * 128 : (mc + 1) * 128],
                            embT_sb[:, kt, sc * 512 : (sc + 1) * 512],
                            start=(kt == 0),
                            stop=(kt == 7),
                        )
                    if mc < 2:  # Q: natural-half eviction per head
                        he, ho = 2 * mc, 2 * mc + 1
                        nc.scalar.activation(
                            out=q2_sb[0:64, he, sc * 512 : (sc + 1) * 512],
                            in_=ps[0:64, :],
                            func=AF.Identity,
                            bias=bqk_sb[0:64, mc : mc + 1],
                            scale=1.0,
                        )
                        nc.scalar.activation(
                            out=q2_sb[64:128, ho, sc * 512 : (sc + 1) * 512],
                            in_=ps[64:128, :],
                            func=AF.Identity,
                            bias=bqk_sb[64:128, mc : mc + 1],
                            scale=1.0,
                        )
                    else:  # K: stage, then scatter into the diag blocks via DMA
                        he, ho = 2 * (mc - 2), 2 * (mc - 2) + 1
                        kstage = work.tile([128, 512], bf16, tag="kstage", bufs=3)
                        nc.scalar.activation(
                            out=kstage[:],
                            in_=ps[:],
                            func=AF.Identity,
                            bias=bqk_sb[:, mc : mc + 1],
                            scale=1.0,
                        )
                        ks = kstage.rearrange("p (k t m) -> p k t m", k=4, m=64)
                        kslc = slice(sc * 4, (sc + 1) * 4)
                        nc.gpsimd.dma_start(
                            out=k2_sb[0:64, he, kslc, 0:64], in_=ks[0:64, :, 0, :]
                        )
                        nc.gpsimd.dma_start(
                            out=k2_sb[64:128, he, kslc, 64:128], in_=ks[0:64, :, 1, :]
                        )
                        nc.gpsimd.dma_start(
                            out=k2_sb[0:64, ho, kslc, 0:64], in_=ks[64:128, :, 0, :]
                        )
                        nc.gpsimd.dma_start(
                            out=k2_sb[64:128, ho, kslc, 64:128], in_=ks[64:128, :, 1, :]
                        )
            # duplicate Q across partition halves (SBUF->SBUF DMA moves partitions)
            for h in range(HL):
                if h % 2 == 0:
                    nc.gpsimd.dma_start(out=q2_sb[64:128, h, :], in_=q2_sb[0:64, h, :])
                else:
                    nc.gpsimd.dma_start(out=q2_sb[0:64, h, :], in_=q2_sb[64:128, h, :])

            # V: [s, (h, a+1)] bf16, ones column LAST per head (sumexp row trick)
            v_sb = persist.tile([128, 16, HL, 1 + A], bf16)
            nc.vector.memset(v_sb, 1.0)
            for st in range(16):
                ps = psA.tile([128, HL * A], f32, tag="aux")
                for kt in range(8):
                    nc.tensor.matmul(
                        ps[:],
                        embT_sb[:, kt, st * 128 : (st + 1) * 128],
                        wv_sb[:, kt, :],
                        start=(kt == 0),
                        stop=False,
                    )
                nc.tensor.matmul(ps[:], ones_b[:, :], bv_sb[:, :], start=False, stop=True)
                nc.vector.tensor_copy(
                    out=v_sb[:, st, :, 0:A],
                    in_=ps.rearrange("p (h a) -> p h a", h=HL),
                )

            # out-proj weights + LN constants issued late: needed only from the
            # first consume (~mid-kernel); keeps them out of the startup DMA rush
            for kc in range(8):
                nc.scalar.dma_start(out=wout_sb[:, kc, :], in_=wout_d[:, kc, :])
            for t, dr in ((gammabc, gamma_d), (betabc, beta_d)):
                src = dr[:, :]
                bc = bass.AP(tensor=src.tensor, offset=src.offset, ap=[[0, 128], src.ap[1]])
                nc.scalar.dma_start(out=t[:], in_=bc)

            # xT: [(a), head pair, s] — heads stacked two per 128 partitions
            xT_sb = persist.tile([128, 2, S], bf16)

            # ---------- attention, software-pipelined two units deep ----------
            def pv_mms(pu, kb0, kb1):
                if pu["ps_x"] is None:
                    pu["ps_x"] = psB.tile([65, 512], f32, name="ps_x", tag="pvx")
                for kb in range(kb0, kb1):
                    nc.tensor.matmul(
                        pu["ps_x"][:],
                        v_sb[:, kb, pu["h"], :],
                        pu["probs"][:, kb, :],
                        start=(kb == 0),
                        stop=(kb == 15),
                    )

            def normalize_evict(pu):
                pq, ph, pps_x = pu["q"], pu["h"], pu["ps_x"]
                qo = pq * 512
                # 1/sumexp: DVE reciprocal is 8 cyc/element/lane, so spread the
                # 512 sums over 64 partitions (8 per lane) before inverting,
                # then gather + broadcast back. DMA hops hide in the 2-unit lag.
                recip = work.tile([65, 512], f32r, tag="recip")
                with nc.allow_low_precision(reason="f32r is bitwise f32"):
                    nc.vector.tensor_copy(out=recip[64:65, :], in_=pps_x[64:65, :])
                se_d = dram.tile([1, 512], f32r, name="sed", tag="sed", bufs=2)
                nc.sync.dma_start(out=se_d[:, :], in_=recip[64:65, :])
                se_sp = work.tile([64, 2, 8], f32r, tag="sesp")
                nc.sync.dma_start(
                    out=se_sp[:, 0, :], in_=se_d.rearrange("o (p j) -> (o p) j", p=64)
                )
                with nc.allow_low_precision(reason="f32r is bitwise f32"):
                    nc.vector.reciprocal(se_sp[:, 1, :], se_sp[:, 0, :])
                recip_d = dram.tile([1, 512], f32r, name="recipd", tag="recipd", bufs=2)
                nc.sync.dma_start(
                    out=recip_d.rearrange("o (p j) -> (o p) j", p=64), in_=se_sp[:, 1, :]
                )
                src = recip_d[:, :]
                bc = bass.AP(tensor=src.tensor, offset=src.offset, ap=[[0, 64], src.ap[-1]])
                nc.sync.dma_start(out=recip[0:64, :], in_=bc)
                if ph % 2 == 0:
                    nc.vector.tensor_tensor(
                        xT_sb[0:64, ph // 2, qo : qo + 512],
                        pps_x[0:64, :],
                        recip[0:64, :],
                        OP.mult,
                    )
                else:
                    xodd = work.tile([64, 512], bf16, tag="xodd")
                    nc.vector.tensor_tensor(
                        xodd[:], pps_x[0:64, :], recip[0:64, :], OP.mult
                    )
                    nc.sync.dma_start(
                        out=xT_sb[64:128, ph // 2, qo : qo + 512], in_=xodd[:]
                    )

            a2a_out = {}
            er_tiles = {}
            # my group rank, runtime: selects my 128-row column block of the
            # gathered x^T when consuming
            g_dyn = nc.sync.partition_id() % 4

            def a2a_quarter(q, pairs=(0, 1)):
                # AllGather x^T within the 4-core group: each core contributes
                # its head-dims (1 or 2 pair-slabs) for all 512 rows of the
                # quarter; everyone receives the gathered dims and consumes
                # only its own 128-row column block. The collective-feeding
                # DMAs ride the idle GPSIMD SWDGE queue so they never sit
                # behind bulk loads in the sync HWDGE FIFO.
                qo = q * 512
                npair = len(pairs)
                key = f"ag{q}_{pairs[0]}"
                in_d = dram.tile([128 * npair, 512], bf16, name=f"i{key}", tag=f"i{key}")
                out_dd = dram.tile([512 * npair, 512], bf16, name=f"o{key}", tag=f"o{key}")
                for j, pr in enumerate(pairs):
                    nc.gpsimd.dma_start(
                        out=in_d[128 * j : 128 * (j + 1), :],
                        in_=xT_sb[:, pr, qo : qo + 512],
                    )
                nc.gpsimd.collective_compute(
                    "AllGather",
                    OP.bypass,
                    replica_groups=GROUPS,
                    ins=[in_d[:, :].opt()],
                    outs=[out_dd[:, :].opt()],
                )
                a2a_out.setdefault(q, []).append((pairs, out_dd))
                if q not in er_tiles:
                    er = work.tile([128, D], f32, tag="er")
                    nc.scalar.dma_start(out=er[:], in_=embres_d[q * 128 : (q + 1) * 128, :])
                    er_tiles[q] = er

            def consume_quarter(q):
                # gathered x^T -> local out-proj (full W_out) -> residual+LN
                xg = work.tile([128, 8, 128], bf16, tag="xg", bufs=1)
                for pairs, out_dd in a2a_out.pop(q):
                    npair = len(pairs)
                    src = out_dd.rearrange("(r p) c -> p r c", p=128)
                    if npair == 2:
                        nc.sync.dma_start(
                            out=xg[:, :, :], in_=src[:, :, bass.ds(g_dyn * 128, 128)]
                        )
                    else:
                        xgv = xg.rearrange("p (k two) c -> p k two c", two=2)
                        nc.sync.dma_start(
                            out=xgv[:, :, pairs[0], :],
                            in_=src[:, :, bass.ds(g_dyn * 128, 128)],
                        )
                er = er_tiles.pop(q)
                y = work.tile([128, D], f32, tag="y")
                stats = work.tile([128, 2, nc.vector.BN_STATS_DIM], f32, tag="stats")
                for dh in range(2):
                    ps_o = psA.tile([128, 512], f32, tag="aux")
                    for kc in range(8):
                        nc.tensor.matmul(
                            ps_o[:],
                            xg[:, kc, :],
                            wout_sb[:, kc, dh * 512 : (dh + 1) * 512],
                            start=(kc == 0),
                            stop=(kc == 7),
                        )
                    hsl = slice(dh * 512, (dh + 1) * 512)
                    nc.vector.tensor_tensor(y[:, hsl], er[:, hsl], ps_o[:], OP.add)
                    nc.vector.bn_stats(out=stats[:, dh, :], in_=y[:, hsl])
                mv = work.tile([128, nc.vector.BN_AGGR_DIM], f32, tag="mv")
                nc.vector.bn_aggr(out=mv[:], in_=stats[:])
                rstd = work.tile([128, 1], f32, tag="rstd")
                nc.scalar.activation(
                    out=rstd[:], in_=mv[:, 1:2], func=AF.Sqrt, bias=eps_sb[:], scale=1.0
                )
                nc.vector.reciprocal(rstd[:], rstd[:])
                nc.vector.tensor_scalar(
                    y[:], y[:], mv[:, 0:1], rstd[:], OP.subtract, OP.mult
                )
                nc.vector.tensor_tensor(y[:], y[:], gammabc[:], OP.mult)
                nc.vector.tensor_tensor(y[:], y[:], betabc[:], OP.add)
                nc.sync.dma_start(out=out_d[q * 128 : (q + 1) * 128, :], in_=y[:])

            def finish(pu):
                normalize_evict(pu)
                q = pu["q"]
                if q == 3:  # split the last gather by head pair: shorter tail
                    if pu["h"] == 1:
                        a2a_quarter(3, pairs=(0,))
                    elif pu["h"] == 3:
                        a2a_quarter(3, pairs=(1,))
                elif pu["h"] == 3:
                    a2a_quarter(q)

            # per-quarter mask slice, double-buffered. One strided dma_start on
            # the ACT HWDGE ring: bulk dependency-free bytes stay off the sync
            # ring (no head-blocking of the small chained transfers) and cost
            # the ACT sequencer a single dispatch.
            maskT_v = maskT_d.rearrange("(kb p) q -> p kb q", p=128)
            mq_tiles = {}

            def load_mask(q):
                mq = work.tile([128, 16, 512], bf16, name="mq", tag="maskq")
                nc.scalar.dma_start(
                    out=mq[:, :, :], in_=maskT_v[:, :, q * 512 : (q + 1) * 512]
                )
                mq_tiles[q] = mq

            load_mask(0)
            units = []
            for quarter in range(4):
                qoff = quarter * 512
                for h in range(4):
                    mq = mq_tiles[quarter]
                    probs = probsp.tile([128, 16, 512], bf16, tag="probs")
                    unit = {"q": quarter, "h": h, "probs": probs, "ps_x": None, "mq": mq}
                    for j in range(8):  # kb pairs
                        ps_s = psS.tile([128, 2, 512], f32, tag="score")
                        for kk in range(2):
                            kb = 2 * j + kk
                            nc.tensor.matmul(
                                ps_s[:, kk, :],
                                k2_sb[:, h, kb, :],
                                q2_sb[:, h, qoff : qoff + 512],
                                start=True,
                                stop=True,
                            )
                        if units:
                            pv_mms(units[-1], 2 * j, 2 * j + 2)
                        nc.scalar.activation(
                            out=probs[:, 2 * j : 2 * j + 2, :],
                            in_=ps_s[:, :, :],
                            func=AF.Exp,
                            scale=0.125,
                        )
                        if j in (3, 7):  # mask applied in 8-kb batches
                            kb0 = 0 if j == 3 else 8
                            nc.vector.tensor_tensor(
                                probs[:, kb0 : kb0 + 8, :],
                                probs[:, kb0 : kb0 + 8, :],
                                mq[:, kb0 : kb0 + 8, :],
                                OP.mult,
                            )
                        if j == 1 and len(units) >= 2:
                            finish(units[-2])
                        if j == 0 and h == 3 and quarter < 3:
                            load_mask(quarter + 1)  # ~1 unit of prefetch lead
                        if j == 5 and quarter >= 2 and h == 1:
                            consume_quarter(quarter - 2)
                    units.append(unit)
            finish(units[-2])
            pv_mms(units[-1], 0, 16)
            finish(units[-1])  # triggers AG(3, pair 1)
            consume_quarter(2)  # overlaps the final gather's latency
            consume_quarter(3)

    nc.compile()
    return nc


def _prep_inputs(embeddings, attention_mask, W_qkv, b_qkv, W_out, b_out, ln_gamma, ln_beta):
    emb = np.asarray(embeddings, dtype=np.float32)
    mask = np.asarray(attention_mask)
    W_qkv = np.asarray(W_qkv, dtype=np.float32)
    b_qkv = np.asarray(b_qkv, dtype=np.float32)
    W_out = np.asarray(W_out, dtype=np.float32)
    b_out = np.asarray(b_out, dtype=np.float32)
    gamma = np.asarray(ln_gamma, dtype=np.float32).reshape(1, D).astype(ml_dtypes.bfloat16)
    beta = np.asarray(ln_beta, dtype=np.float32).reshape(1, D).astype(ml_dtypes.bfloat16)

    woutF = np.ascontiguousarray(
        W_out.reshape(8, 128, D).transpose(1, 0, 2)
    ).astype(ml_dtypes.bfloat16)

    in_maps = []
    for c in range(NCORES):
        b = c // G
        g = c % G
        hs = g * HL * A  # 256g
        embT = np.ascontiguousarray(emb[b].T).astype(ml_dtypes.bfloat16)
        maskT = np.ascontiguousarray(mask[b].T).astype(ml_dtypes.bfloat16)
        wqk = np.ascontiguousarray(
            np.concatenate([W_qkv[:, hs : hs + 256], W_qkv[:, D + hs : D + hs + 256]], axis=1)
        ).astype(ml_dtypes.bfloat16)
        wv = np.ascontiguousarray(W_qkv[:, 2 * D + hs : 2 * D + hs + 256]).astype(
            ml_dtypes.bfloat16
        )
        bqk = np.concatenate([b_qkv[hs : hs + 256], b_qkv[D + hs : D + hs + 256]])
        bqk = np.ascontiguousarray(bqk.reshape(4, 128).T)
        bv = np.ascontiguousarray(
            b_qkv[2 * D + hs : 2 * D + hs + 256].reshape(1, 256)
        ).astype(ml_dtypes.bfloat16)
        embres = np.concatenate(
            [emb[b, 512 * q + 128 * g : 512 * q + 128 * g + 128, :] for q in range(4)],
            axis=0,
        ) + b_out.reshape(1, D)
        in_maps.append(
            {
                "embT": embT,
                "embres": np.ascontiguousarray(embres),
                "maskT": maskT,
                "wqk": wqk,
                "wv": wv,
                "bqk": bqk,
                "bv": bv,
                "onesb": np.ones((1, 128), dtype=ml_dtypes.bfloat16),
                "wout": woutF,
                "gamma": gamma,
                "beta": beta,
            }
        )
    return in_maps


def _run(inputs, trace=False, **kw):
    if "nc" not in _CACHE:
        _CACHE["nc"] = _build()
    nc = _CACHE["nc"]
    in_maps = _prep_inputs(**inputs)
    res = run_bass_kernel_spmd(nc, in_maps, list(range(NCORES)), trace=trace, **kw)
    out = np.empty((B, S, D), dtype=np.float32)
    for c in range(NCORES):
        b, g = c // G, c % G
        for q in range(4):
            out[b, 512 * q + 128 * g : 512 * q + 128 * g + 128, :] = res.results[c][
                "out"
            ][128 * q : 128 * (q + 1), :]
    return out, res


def kernel(**inputs):
    out, _ = _run(inputs, trace=False)
    return out


# revision 36
# speedup vs baseline: 1.2306x; 1.0007x over previous
"""Multi-head self-attention block (B=2, S=2048, D=1024, H=16) on 8 TRN2 cores.

Sharding: 2-way data-parallel over batch x 4-way tensor-parallel over heads.
Core c handles batch b=c//4 with group rank g=c%4 (heads 4g..4g+4). After
attention, the per-head outputs x^T are exchanged with one small AllToAll
per q-quarter over the 4-core batch group (pure permutation, 4x fewer bytes
than reduce-scattering out-proj partials and no CC ALU work); each core then
runs the out-projection locally with the full W_out for the 128 output rows
it owns per quarter (rows 512q + 128g .. +128), adds the residual (with
b_out pre-folded on host) and LayerNorms. Collectives overlap the remaining
attention quarters; only quarter 3's exchange lands on the tail.

Score matmuls are packed block-diagonally (two 64-wide k-halves on the
128 partitions with Q duplicated) so the PE array runs fully active.

Self-contained: hardcodes all shapes; builds the Bass program once.
"""

import os
import sys

sys.path.insert(0, "/opt/trn_rl_repo")

import numpy as np
import ml_dtypes

import concourse.bass as bass
import concourse.tile as tile
from concourse import bacc, mybir
from concourse.bass_utils import run_bass_kernel_spmd

B, S, D, H = 2, 2048, 1024, 16
A = D // H  # 64
NCORES = 8
G = 4  # cores per batch group
HL = H // G  # local heads per core = 4
M_QK = 2 * HL * A  # 512 rows of Q_T+K_T per core
QB = S // G  # 512
EPS = 1e-3
GROUPS = [[0, 1, 2, 3], [4, 5, 6, 7]]

f32 = mybir.dt.float32
f32r = mybir.dt.float32r
bf16 = mybir.dt.bfloat16

AF = mybir.ActivationFunctionType
OP = mybir.AluOpType

_CACHE = {}


def _build():
    nc = bacc.Bacc("TRN2", target_bir_lowering=False, debug=False, num_devices=NCORES)

    # ---- I/O ----
    embT_d = nc.dram_tensor("embT", [D, S], bf16, kind="ExternalInput")
    embres_d = nc.dram_tensor("embres", [QB, D], f32, kind="ExternalInput")
    maskT_d = nc.dram_tensor("maskT", [S, S], bf16, kind="ExternalInput")
    wqk_d = nc.dram_tensor("wqk", [D, M_QK], bf16, kind="ExternalInput")
    wv_d = nc.dram_tensor("wv", [D, HL * A], bf16, kind="ExternalInput")
    bqk_d = nc.dram_tensor("bqk", [128, 4], f32, kind="ExternalInput")
    bv_d = nc.dram_tensor("bv", [1, HL * A], bf16, kind="ExternalInput")
    onesb_d = nc.dram_tensor("onesb", [1, 128], bf16, kind="ExternalInput")
    wout_d = nc.dram_tensor("wout", [128, 8, D], bf16, kind="ExternalInput")
    gamma_d = nc.dram_tensor("gamma", [1, D], bf16, kind="ExternalInput")
    beta_d = nc.dram_tensor("beta", [1, D], bf16, kind="ExternalInput")
    out_d = nc.dram_tensor("out", [QB, D], f32, kind="ExternalOutput")

    with tile.TileContext(nc) as tc:
        with (
            tc.tile_pool(name="big", bufs=1) as big,
            tc.tile_pool(name="persist", bufs=1) as persist,
            tc.tile_pool(name="probs", bufs=2) as probsp,
            tc.tile_pool(name="work", bufs=2) as work,
            tc.tile_pool(name="psA", bufs=2, space="PSUM") as psA,  # 1-bank f32 mm
            tc.tile_pool(name="psS", bufs=2, space="PSUM") as psS,  # scores (2 banks)
            tc.tile_pool(name="psB", bufs=2, space="PSUM") as psB,  # pv xT (1 bank)
            tc.tile_pool(name="dram", bufs=1, space="DRAM") as dram,
        ):
            # ---------- constants / small weights first (cheap, unblock evictions) ----------
            bqk_sb = persist.tile([128, 4], f32)
            nc.sync.dma_start(out=bqk_sb, in_=bqk_d[:, :])
            bv_sb = persist.tile([1, HL * A], bf16)
            nc.sync.dma_start(out=bv_sb, in_=bv_d[:, :])
            ones_b = persist.tile([1, 128], bf16)
            nc.sync.dma_start(out=ones_b, in_=onesb_d[:, :])

            # ---------- embT + wqk interleaved, s-quarter-major: the first
            # QKV matmul group (sc=0) only needs embT[:, :, 0:512] + wqk.
            embT_sb = big.tile([128, 8, S], bf16, tag="bigslot")
            wqk_sb = persist.tile([128, 8, M_QK], bf16)
            wv_sb = persist.tile([128, 8, HL * A], bf16)
            for kt in range(8):
                nc.sync.dma_start(
                    out=embT_sb[:, kt, 0:512], in_=embT_d[kt * 128 : (kt + 1) * 128, 0:512]
                )
                nc.sync.dma_start(out=wqk_sb[:, kt, :], in_=wqk_d[kt * 128 : (kt + 1) * 128, :])
            for kt in range(8):
                nc.sync.dma_start(
                    out=embT_sb[:, kt, 512:S], in_=embT_d[kt * 128 : (kt + 1) * 128, 512:S]
                )
            for kt in range(8):
                nc.sync.dma_start(out=wv_sb[:, kt, :], in_=wv_d[kt * 128 : (kt + 1) * 128, :])

            wout_sb = persist.tile([128, 8, D], bf16)
            eps_sb = persist.tile([128, 1], f32)
            nc.vector.memset(eps_sb, EPS)
            gammabc = persist.tile([128, D], bf16)
            betabc = persist.tile([128, D], bf16)

            # per-quarter mask slices, double-buffered; one strided dma_start
            # each on the ACT HWDGE ring so the bulk bytes stay off the sync
            # ring. mask(0) is kicked off mid-QKV so it lands before unit 0.
            maskT_v = maskT_d.rearrange("(kb p) q -> p kb q", p=128)
            mq_tiles = {}

            def load_mask(q):
                mq = work.tile([128, 16, 512], bf16, name="mq", tag="maskq")
                nc.scalar.dma_start(
                    out=mq[:, :, :], in_=maskT_v[:, :, q * 512 : (q + 1) * 512]
                )
                mq_tiles[q] = mq

            # ---------- QKV projection ----------
            # Q duplicated on both partition halves: q2[p, h, s], p<64 and
            # p>=64 both hold Q_h[p % 64, s].
            q2_sb = persist.tile([128, HL, S], bf16)
            # K block-diag: k2[0:64, h, kb, 0:64] = K_h[a, 128 kb + m],
            # k2[64:128, h, kb, 64:128] = K_h[a, 128 kb + 64 + m], zeros off-diag.
            k2_sb = persist.tile([128, HL, 16, 128], bf16)
            nc.vector.memset(k2_sb, 0.0)

            for sc in range(4):  # sc-outer so the first group needs only embT sc=0
                if sc == 2:
                    load_mask(0)
                for mc in range(4):  # 0: Q h0-1, 1: Q h2-3, 2: K h0-1, 3: K h2-3
                    ps = psA.tile([128, 512], f32, tag="aux")
                    for kt in range(8):
                        nc.tensor.matmul(
                            ps[:],
                            wqk_sb[:, kt, mc * 128 : (mc + 1) * 128],
                            embT_sb[:, kt, sc * 512 : (sc + 1) * 512],
                            start=(kt == 0),
                            stop=(kt == 7),
                        )
                    if mc < 2:  # Q: natural-half eviction per head
                        he, ho = 2 * mc, 2 * mc + 1
                        nc.scalar.activation(
                            out=q2_sb[0:64, he, sc * 512 : (sc + 1) * 512],
                            in_=ps[0:64, :],
                            func=AF.Identity,
                            bias=bqk_sb[0:64, mc : mc + 1],
                            scale=1.0,
                        )
                        nc.scalar.activation(
                            out=q2_sb[64:128, ho, sc * 512 : (sc + 1) * 512],
                            in_=ps[64:128, :],
                            func=AF.Identity,
                            bias=bqk_sb[64:128, mc : mc + 1],
                            scale=1.0,
                        )
                    else:  # K: stage, then scatter into the diag blocks via DMA
                        he, ho = 2 * (mc - 2), 2 * (mc - 2) + 1
                        kstage = work.tile([128, 512], bf16, tag="kstage", bufs=3)
                        nc.scalar.activation(
                            out=kstage[:],
                            in_=ps[:],
                            func=AF.Identity,
                            bias=bqk_sb[:, mc : mc + 1],
                            scale=1.0,
                        )
                        ks = kstage.rearrange("p (k t m) -> p k t m", k=4, m=64)
                        kslc = slice(sc * 4, (sc + 1) * 4)
                        nc.gpsimd.dma_start(
                            out=k2_sb[0:64, he, kslc, 0:64], in_=ks[0:64, :, 0, :]
                        )
                        nc.gpsimd.dma_start(
                            out=k2_sb[64:128, he, kslc, 64:128], in_=ks[0:64, :, 1, :]
                        )
                        nc.gpsimd.dma_start(
                            out=k2_sb[0:64, ho, kslc, 0:64], in_=ks[64:128, :, 0, :]
                        )
                        nc.gpsimd.dma_start(
                            out=k2_sb[64:128, ho, kslc, 64:128], in_=ks[64:128, :, 1, :]
                        )
            # duplicate Q across partition halves (SBUF->SBUF DMA moves partitions)
            for h in range(HL):
                if h % 2 == 0:
                    nc.gpsimd.dma_start(out=q2_sb[64:128, h, :], in_=q2_sb[0:64, h, :])
                else:
                    nc.gpsimd.dma_start(out=q2_sb[0:64, h, :], in_=q2_sb[64:128, h, :])

            # V: [s, (h, a+1)] bf16, ones column LAST per head (sumexp row trick)
            v_sb = persist.tile([128, 16, HL, 1 + A], bf16)
            nc.vector.memset(v_sb, 1.0)
            for st in range(16):
                ps = psA.tile([128, HL * A], f32, tag="aux")
                for kt in range(8):
                    nc.tensor.matmul(
                        ps[:],
                        embT_sb[:, kt, st * 128 : (st + 1) * 128],
                        wv_sb[:, kt, :],
                        start=(kt == 0),
                        stop=False,
                    )
                nc.tensor.matmul(ps[:], ones_b[:, :], bv_sb[:, :], start=False, stop=True)
                nc.vector.tensor_copy(
                    out=v_sb[:, st, :, 0:A],
                    in_=ps.rearrange("p (h a) -> p h a", h=HL),
                )

            # out-proj weights + LN constants issued late: needed only from the
            # first consume (~mid-kernel); keeps them out of the startup DMA rush
            for kc in range(8):
                nc.sync.dma_start(out=wout_sb[:, kc, :], in_=wout_d[:, kc, :])
            for t, dr in ((gammabc, gamma_d), (betabc, beta_d)):
                src = dr[:, :]
                bc = bass.AP(tensor=src.tensor, offset=src.offset, ap=[[0, 128], src.ap[1]])
                nc.sync.dma_start(out=t[:], in_=bc)

            # xT: [(a), head pair, s] — heads stacked two per 128 partitions
            xT_sb = persist.tile([128, 2, S], bf16)

            # ---------- attention, software-pipelined two units deep ----------
            def pv_mms(pu, kb0, kb1):
                if pu["ps_x"] is None:
                    pu["ps_x"] = psB.tile([65, 512], f32, name="ps_x", tag="pvx")
                for kb in range(kb0, kb1):
                    nc.tensor.matmul(
                        pu["ps_x"][:],
                        v_sb[:, kb, pu["h"], :],
                        pu["probs"][:, kb, :],
                        start=(kb == 0),
                        stop=(kb == 15),
                    )

            def normalize_evict(pu):
                pq, ph, pps_x = pu["q"], pu["h"], pu["ps_x"]
                qo = pq * 512
                # 1/sumexp: DVE reciprocal is 8 cyc/element/lane, so spread the
                # 512 sums over 64 partitions (8 per lane) before inverting,
                # then gather + broadcast back. DMA hops hide in the 2-unit lag.
                recip = work.tile([65, 512], f32r, tag="recip")
                with nc.allow_low_precision(reason="f32r is bitwise f32"):
                    nc.vector.tensor_copy(out=recip[64:65, :], in_=pps_x[64:65, :])
                se_d = dram.tile([1, 512], f32r, name="sed", tag="sed", bufs=2)
                nc.sync.dma_start(out=se_d[:, :], in_=recip[64:65, :])
                se_sp = work.tile([64, 2, 8], f32r, tag="sesp")
                nc.sync.dma_start(
                    out=se_sp[:, 0, :], in_=se_d.rearrange("o (p j) -> (o p) j", p=64)
                )
                with nc.allow_low_precision(reason="f32r is bitwise f32"):
                    nc.vector.reciprocal(se_sp[:, 1, :], se_sp[:, 0, :])
                recip_d = dram.tile([1, 512], f32r, name="recipd", tag="recipd", bufs=2)
                nc.sync.dma_start(
                    out=recip_d.rearrange("o (p j) -> (o p) j", p=64), in_=se_sp[:, 1, :]
                )
                src = recip_d[:, :]
                bc = bass.AP(tensor=src.tensor, offset=src.offset, ap=[[0, 64], src.ap[-1]])
                nc.sync.dma_start(out=recip[0:64, :], in_=bc)
                if ph % 2 == 0:
                    nc.vector.tensor_tensor(
                        xT_sb[0:64, ph // 2, qo : qo + 512],
                        pps_x[0:64, :],
                        recip[0:64, :],
                        OP.mult,
                    )
                else:
                    xodd = work.tile([64, 512], bf16, tag="xodd")
                    nc.vector.tensor_tensor(
                        xodd[:], pps_x[0:64, :], recip[0:64, :], OP.mult
                    )
                    nc.sync.dma_start(
                        out=xT_sb[64:128, ph // 2, qo : qo + 512], in_=xodd[:]
                    )

            a2a_out = {}
            er_tiles = {}
            # my group rank, runtime: selects my 128-row column block of the
            # gathered x^T when consuming
            g_dyn = nc.sync.partition_id() % 4

            def a2a_quarter(q, pairs=(0, 1)):
                # AllGather x^T within the 4-core group: each core contributes
                # its head-dims (1 or 2 pair-slabs) for all 512 rows of the
                # quarter; everyone receives the gathered dims and consumes
                # only its own 128-row column block. The collective-feeding
                # DMAs ride the idle GPSIMD SWDGE queue so they never sit
                # behind bulk loads in the sync HWDGE FIFO.
                qo = q * 512
                npair = len(pairs)
                key = f"ag{q}_{pairs[0]}"
                in_d = dram.tile([128 * npair, 512], bf16, name=f"i{key}", tag=f"i{key}")
                out_dd = dram.tile([512 * npair, 512], bf16, name=f"o{key}", tag=f"o{key}")
                for j, pr in enumerate(pairs):
                    nc.gpsimd.dma_start(
                        out=in_d[128 * j : 128 * (j + 1), :],
                        in_=xT_sb[:, pr, qo : qo + 512],
                    )
                nc.gpsimd.collective_compute(
                    "AllGather",
                    OP.bypass,
                    replica_groups=GROUPS,
                    ins=[in_d[:, :].opt()],
                    outs=[out_dd[:, :].opt()],
                )
                a2a_out.setdefault(q, []).append((pairs, out_dd))
                if q not in er_tiles:
                    er = work.tile([128, D], f32, tag="er")
                    nc.scalar.dma_start(out=er[:], in_=embres_d[q * 128 : (q + 1) * 128, :])
                    er_tiles[q] = er

            def consume_quarter(q):
                # gathered x^T -> local out-proj (full W_out) -> residual+LN
                xg = work.tile([128, 8, 128], bf16, tag="xg", bufs=1)
                for pairs, out_dd in a2a_out.pop(q):
                    npair = len(pairs)
                    src = out_dd.rearrange("(r p) c -> p r c", p=128)
                    if npair == 2:
                        nc.sync.dma_start(
                            out=xg[:, :, :], in_=src[:, :, bass.ds(g_dyn * 128, 128)]
                        )
                    else:
                        xgv = xg.rearrange("p (k two) c -> p k two c", two=2)
                        nc.sync.dma_start(
                            out=xgv[:, :, pairs[0], :],
                            in_=src[:, :, bass.ds(g_dyn * 128, 128)],
                        )
                er = er_tiles.pop(q)
                y = work.tile([128, D], f32, tag="y")
                stats = work.tile([128, 2, nc.vector.BN_STATS_DIM], f32, tag="stats")
                for dh in range(2):
                    ps_o = psA.tile([128, 512], f32, tag="aux")
                    for kc in range(8):
                        nc.tensor.matmul(
                            ps_o[:],
                            xg[:, kc, :],
                            wout_sb[:, kc, dh * 512 : (dh + 1) * 512],
                            start=(kc == 0),
                            stop=(kc == 7),
                        )
                    hsl = slice(dh * 512, (dh + 1) * 512)
                    nc.vector.tensor_tensor(y[:, hsl], er[:, hsl], ps_o[:], OP.add)
                    nc.vector.bn_stats(out=stats[:, dh, :], in_=y[:, hsl])
                mv = work.tile([128, nc.vector.BN_AGGR_DIM], f32, tag="mv")
                nc.vector.bn_aggr(out=mv[:], in_=stats[:])
                rstd = work.tile([128, 1], f32, tag="rstd")
                nc.scalar.activation(
                    out=rstd[:], in_=mv[:, 1:2], func=AF.Sqrt, bias=eps_sb[:], scale=1.0
                )
                nc.vector.reciprocal(rstd[:], rstd[:])
                nc.vector.tensor_scalar(
                    y[:], y[:], mv[:, 0:1], rstd[:], OP.subtract, OP.mult
                )
                nc.vector.tensor_tensor(y[:], y[:], gammabc[:], OP.mult)
                nc.vector.tensor_tensor(y[:], y[:], betabc[:], OP.add)
                nc.sync.dma_start(out=out_d[q * 128 : (q + 1) * 128, :], in_=y[:])

            def finish(pu):
                normalize_evict(pu)
                q = pu["q"]
                if q == 3:  # split the last gather by head pair: shorter tail
                    if pu["h"] == 1:
                        a2a_quarter(3, pairs=(0,))
                    elif pu["h"] == 3:
                        a2a_quarter(3, pairs=(1,))
                elif pu["h"] == 3:
                    a2a_quarter(q)

            units = []
            for quarter in range(4):
                qoff = quarter * 512
                for h in range(4):
                    mq = mq_tiles[quarter]
                    probs = probsp.tile([128, 16, 512], bf16, tag="probs")
                    unit = {"q": quarter, "h": h, "probs": probs, "ps_x": None, "mq": mq}
                    for j in range(8):  # kb pairs
                        ps_s = psS.tile([128, 2, 512], f32, tag="score")
                        for kk in range(2):
                            kb = 2 * j + kk
                            nc.tensor.matmul(
                                ps_s[:, kk, :],
                                k2_sb[:, h, kb, :],
                                q2_sb[:, h, qoff : qoff + 512],
                                start=True,
                                stop=True,
                            )
                        if units:
                            pv_mms(units[-1], 2 * j, 2 * j + 2)
                        nc.scalar.activation(
                            out=probs[:, 2 * j : 2 * j + 2, :],
                            in_=ps_s[:, :, :],
                            func=AF.Exp,
                            scale=0.125,
                        )
                        if j in (3, 7):  # mask applied in 8-kb batches
                            kb0 = 0 if j == 3 else 8
                            nc.vector.tensor_tensor(
                                probs[:, kb0 : kb0 + 8, :],
                                probs[:, kb0 : kb0 + 8, :],
                                mq[:, kb0 : kb0 + 8, :],
                                OP.mult,
                            )
                        if j == 1 and len(units) >= 2:
                            finish(units[-2])
                        if j == 0 and h == 3 and quarter < 3:
                            load_mask(quarter + 1)  # ~1 unit of prefetch lead
                        if j == 5 and quarter >= 2 and h == 1:
                            consume_quarter(quarter - 2)
                    units.append(unit)
            finish(units[-2])
            pv_mms(units[-1], 0, 16)
            finish(units[-1])  # triggers AG(3, pair 1)
            consume_quarter(2)  # overlaps the final gather's latency
            consume_quarter(3)

    nc.compile()
    return nc


def _prep_inputs(embeddings, attention_mask, W_qkv, b_qkv, W_out, b_out, ln_gamma, ln_beta):
    emb = np.asarray(embeddings, dtype=np.float32)
    mask = np.asarray(attention_mask)
    W_qkv = np.asarray(W_qkv, dtype=np.float32)
    b_qkv = np.asarray(b_qkv, dtype=np.float32)
    W_out = np.asarray(W_out, dtype=np.float32)
    b_out = np.asarray(b_out, dtype=np.float32)
    gamma = np.asarray(ln_gamma, dtype=np.float32).reshape(1, D).astype(ml_dtypes.bfloat16)
    beta = np.asarray(ln_beta, dtype=np.float32).reshape(1, D).astype(ml_dtypes.bfloat16)

    woutF = np.ascontiguousarray(
        W_out.reshape(8, 128, D).transpose(1, 0, 2)
    ).astype(ml_dtypes.bfloat16)

    in_maps = []
    for c in range(NCORES):
        b = c // G
        g = c % G
        hs = g * HL * A  # 256g
        embT = np.ascontiguousarray(emb[b].T).astype(ml_dtypes.bfloat16)
        maskT = np.ascontiguousarray(mask[b].T).astype(ml_dtypes.bfloat16)
        wqk = np.ascontiguousarray(
            np.concatenate([W_qkv[:, hs : hs + 256], W_qkv[:, D + hs : D + hs + 256]], axis=1)
        ).astype(ml_dtypes.bfloat16)
        wv = np.ascontiguousarray(W_qkv[:, 2 * D + hs : 2 * D + hs + 256]).astype(
            ml_dtypes.bfloat16
        )
        bqk = np.concatenate([b_qkv[hs : hs + 256], b_qkv[D + hs : D + hs + 256]])
        bqk = np.ascontiguousarray(bqk.reshape(4, 128).T)
        bv = np.ascontiguousarray(
            b_qkv[2 * D + hs : 2 * D + hs + 256].reshape(1, 256)
        ).astype(ml_dtypes.bfloat16)
        embres = np.concatenate(
            [emb[b, 512 * q + 128 * g : 512 * q + 128 * g + 128, :] for q in range(4)],
            axis=0,
        ) + b_out.reshape(1, D)
        in_maps.append(
            {
                "embT": embT,
                "embres": np.ascontiguousarray(embres),
                "maskT": maskT,
                "wqk": wqk,
                "wv": wv,
                "bqk": bqk,
                "bv": bv,
                "onesb": np.ones((1, 128), dtype=ml_dtypes.bfloat16),
                "wout": woutF,
                "gamma": gamma,
                "beta": beta,
            }
        )
    return in_maps


def _run(inputs, trace=False, **kw):
    if "nc" not in _CACHE:
        _CACHE["nc"] = _build()
    nc = _CACHE["nc"]
    in_maps = _prep_inputs(**inputs)
    res = run_bass_kernel_spmd(nc, in_maps, list(range(NCORES)), trace=trace, **kw)
    out = np.empty((B, S, D), dtype=np.float32)
    for c in range(NCORES):
        b, g = c // G, c % G
        for q in range(4):
            out[b, 512 * q + 128 * g : 512 * q + 128 * g + 128, :] = res.results[c][
                "out"
            ][128 * q : 128 * (q + 1), :]
    return out, res


def kernel(**inputs):
    out, _ = _run(inputs, trace=False)
    return out
